# revision 34
# baseline (speedup 1.0000x reference)
"""Trainium2 Bass kernel for nn_Graph_to_Featuremaps_savemem.

Reference computation:
    scores[b,p,n] = s_res[b,p] + s_hid[b,n];  attn = softmax_n(scores)
    out[b,c,p]    = relu(sum_n attn[b,p,n] * (x[b,n,:] @ W)[c])

Key simplification: softmax over n is shift-invariant, so the per-pixel
s_res[b,p] term (the only use of res_feature / node_fea_for_res) cancels:
    attn[b,p,n] = softmax_n(s_hid[b,n])   (independent of p)
    out[b,c,p]  = relu(sum_n a[b,n] * nv[b,n,c])  broadcast over all pixels.

So the kernel is a tiny softmax-weighted matmul per batch followed by a
151 MB broadcast-write of the (B, C) result over H*W pixels. Sharding:
data-parallel over batch, 2 batches per core across 8 cores; the small
params (node_fea_for_hidden, weight) are replicated.

The structure targets the DMA-store roofline (16 DMA engines x ~27 GB/s
per core): the 18.9 MB/core output stream goes on the wire at ~15 us
(bounded by the serial dependence chain through the input DMA, exp, two
tiny matmul rounds and the first 1.4 us chunk fill) and streams at line
rate for ~44 us with every other cost hidden under it; ~61 us total vs
the 79 us baseline.  Structural choices, each verified against perfetto
traces:

  - Stores are PLAIN DMAs from a fully materialized (128, 2, P) image per
    batch.  Stride-0 "replicate the chunk" source patterns would avoid
    materializing, but they perturb DMA engine E79 (which also hosts the
    queues' descriptor generation) and stretch the stream tail 4-10 us.
  - The stream still starts early because batch0/c-low is stored in three
    pixel slices (2304 / 2304 / 4608 wide) triggered as their fills land.
  - DVE fills the two c-low halves (tensor_scalar mult+max fuses the
    1/denom scale and the ReLU into the broadcast), the scalar engine
    concurrently fills the two c-high halves (activation Relu with
    per-partition scale).  v and 1/denom are funneled to SBUF on DVE so
    every fill and every DMA trigger carries at most ONE sync wait (HW
    limit: matmul / tensor-scalar / DMA-trigger have a single wait slot).
  - Matmul operands (x, W, ones) are packed host-side as bf16 pairs
    inside the f32 input payloads and read via bitcast APs: no DVE cast
    instructions exist, so the scheduler cannot hoist weight casts into
    the critical exp -> rhs_e -> y -> v chain (its DMA-time model is
    wrong and it otherwise does exactly that).
  - A dummy K=14 matmul reading both input tiles runs first on the PE:
    its single queue-semaphore wait transitively covers all later PE
    reads of DMA-produced operands, letting them pair with DVE-produced
    operands within the one-wait budget.
  - Both input DMAs ride the sync queue, tiny x part first (concurrent
    triggers on two queues contend in the descriptor generator and delay
    x by ~1 us); s_hid = x . nfh is a DVE multiply + reduce against a
    host-packed nfh replica -- no PE transposes anywhere.
  - softmax normalization is deferred: y = x^T (mask * exp(s)) and
    v = W^T y use unnormalized weights; 1/denom rides the fills.
  - bf16 matmuls (O(1) gaussian data; tolerance 2e-2, measured ~4e-3).
  - The kernel-tail drain is stripped to the final output DMA's
    completion semaphore (queue FIFO makes it dominate everything).
"""

import numpy as np

import concourse.bass as bass
import concourse.mybir as mybir
import concourse.tile as tile
from concourse.bass_utils import run_bass_kernel_spmd

B, NODES, HID, C, H, W = 16, 7, 256, 256, 96, 96
P = H * W                # 9216 pixels
NCORES = 8
BL = B // NCORES         # 2 local batches per core
BN = BL * NODES          # 14 (b,n) rows
# Pixel slices of the first (b0, c-low) store.  Descriptor width floor:
# 9.2 KB (2304 px) descriptors are tolerated, but 4.6 KB ones push the
# descriptor generator into a lagging regime that periodically steals
# ~35% of DMA engine E79's bandwidth for the REST of the stream (~9 us on
# the tail, traced empirically).  Total DMA count must stay <= 8 or Tile
# runs out of single-slot sync waits on the later stores.
CUTS = [0, 2304, 4608, P]

# cin_a layout (f32, rows 0:14): x f32, nfh replica, blockmask, then
# bf16-pair-packed regions (ones column, x, ones row) read via bitcast.
COL_X = 0
COL_NFH = 256
COL_BM = 512
COL_ONEC = 514           # 1 col:  bf16 [1.0, -] per row
COL_XBF = 515            # 128 cols: x as bf16 pairs
COL_ONER = 643           # 64 cols, row 0: (1, 128) bf16 ones row
CINA_COLS = 707
# cin_b: W as bf16 pairs, [k, (kh*256 + c)/2] f32 words (k = h % 128).
CINB_COLS = C

_cache: dict = {}


def _rep_ap(ap, dims):
    """Return a copy of `ap` with its non-partition dims replaced by `dims`
    (list of [stride, count]); used to build stride-0 broadcast patterns."""
    a = ap.copy()
    a.ap = mybir.VecI64Pair([list(a.ap[0])] + [list(d) for d in dims])
    return a


def _build_nc():
    nc = bass.Bass()
    f32 = mybir.dt.float32
    bf16 = mybir.dt.bfloat16
    cina_d = nc.declare_dram_parameter("cina", [128, CINA_COLS], f32, isOutput=False)
    cinb_d = nc.declare_dram_parameter("cinb", [128, CINB_COLS], f32, isOutput=False)
    out_d = nc.declare_dram_parameter("out", [BL, C, P], f32, isOutput=True)

    with tile.TileContext(nc) as tc:
        with (
            tc.tile_pool(name="sb", bufs=1) as sb,
            tc.tile_pool(name="ps", bufs=1, space=bass.MemorySpace.PSUM) as ps,
        ):
            cina = sb.tile([128, CINA_COLS], f32)
            cinb = sb.tile([128, CINB_COLS], f32)
            # Both input loads on the sync queue, tiny x part first
            # (concurrent triggers on two queues contend in the shared
            # descriptor generator and delay the critical x load ~1 us;
            # triggering x from the scalar engine was also tried and its
            # first-trigger cost proved erratic, up to 1.5 us).
            nc.sync.dma_start(out=cina[0:BN, :], in_=cina_d[0:BN, :])
            nc.sync.dma_start(out=cinb[:], in_=cinb_d[:])

            x_sl = cina[0:BN, COL_X : COL_X + HID]
            nfh_sl = cina[0:BN, COL_NFH : COL_NFH + HID]
            bm_sl = cina[0:BN, COL_BM : COL_BM + BL]
            ones_col = cina[0:BN, COL_ONEC : COL_ONEC + 1].bitcast(bf16)[:, 0:1]
            x_bf = cina[0:BN, COL_XBF : COL_XBF + HID // 2].bitcast(bf16)
            ones_row = cina[0:1, COL_ONER : COL_ONER + 64].bitcast(bf16)

            def w_bf(kh, ch):
                lo = kh * 128 + ch * 64
                return cinb[:, lo : lo + 64].bitcast(bf16)

            # s[(b n)] = sum_h x * nfh, fused into one DVE pass
            # (scalar_tensor_tensor with accum_out; the dedicated
            # tensor_tensor_reduce fails walrus codegen in this toolchain).
            tt_scratch = sb.tile([128, HID], f32)
            s_col = sb.tile([128, 1], f32)
            nc.vector.scalar_tensor_tensor(
                out=tt_scratch[0:BN, :], in0=x_sl, scalar=1.0, in1=nfh_sl,
                op0=mybir.AluOpType.bypass, op1=mybir.AluOpType.mult,
                accum_out=s_col[0:BN, :],
            )
            # e = exp(s) on the scalar engine (normalization deferred).
            e_col = sb.tile([128, 1], f32)
            nc.scalar.activation(
                e_col[0:BN, :], s_col[0:BN, :], mybir.ActivationFunctionType.Exp
            )
            # rhs_e[(b n), b'] = blockmask * e  (unnormalized attn weights).
            rhs_e = sb.tile([128, BL], bf16)
            nc.vector.tensor_scalar(
                out=rhs_e[0:BN, :], in0=bm_sl, scalar1=e_col[0:BN, 0:1],
                scalar2=None, op0=mybir.AluOpType.mult,
            )

            # Two dummy matmuls, one per input queue: their queue-sem
            # waits transitively cover every later PE read of DMA-produced
            # operands (single-wait-slot rule; one wait each).
            ps_junk = ps.tile([1, 2], f32, tag="junk")
            nc.tensor.matmul(
                ps_junk[:], ones_col, x_bf[:, 0:2], start=True, stop=True
            )
            nc.tensor.matmul(
                ps_junk[:], ones_col, cinb[0:BN, 0:1].bitcast(bf16),
                start=True, stop=True,
            )
            # denom[b] = sum_n e ; y[h, b] = sum_n x * e (contract over bn).
            ps_den = ps.tile([1, BL], f32, tag="den")
            nc.tensor.matmul(ps_den[:], ones_col, rhs_e[0:BN, :], start=True, stop=True)
            ps_y = ps.tile([128, 2 * BL], f32, tag="y")
            for kh in range(2):
                nc.tensor.matmul(
                    ps_y[:, kh * BL : (kh + 1) * BL],
                    x_bf[:, kh * 128 : (kh + 1) * 128],
                    rhs_e[0:BN, :],
                    start=True, stop=True,
                )
            recip = sb.tile([1, BL], bf16)
            with nc.allow_low_precision(reason="1/denom in bf16; tol 2e-2"):
                nc.vector.reciprocal(recip[:], ps_den[:])
            s_y = sb.tile([128, 2 * BL], bf16)
            nc.vector.tensor_copy(out=s_y[:], in_=ps_y[:])

            # v[c, b] = sum_h W[h, c] * y[h, b]   (c-half per group).
            ps_v = ps.tile([128, 2 * BL], f32, tag="v")
            for ch in range(2):
                for kh in range(2):
                    nc.tensor.matmul(
                        ps_v[:, ch * BL : (ch + 1) * BL],
                        w_bf(kh, ch),
                        s_y[:, kh * BL : (kh + 1) * BL],
                        start=(kh == 0), stop=(kh == 1),
                    )
            # Broadcast 1/denom to all partitions with a K=1 matmul, after
            # the v matmuls so its reciprocal wait never stalls them, then
            # funnel v and 1/denom to SBUF on DVE.
            ps_r = ps.tile([128, BL], f32, tag="r")
            nc.tensor.matmul(ps_r[:], ones_row, recip[:], start=True, stop=True)
            s_v = sb.tile([128, 2 * BL], f32)
            nc.vector.tensor_copy(out=s_v[:], in_=ps_v[:])
            s_rr = sb.tile([128, BL], f32)
            nc.vector.tensor_copy(out=s_rr[:], in_=ps_r[:])

            # Normalize + ReLU + materialize the broadcast, store with
            # plain DMAs (see module docstring for why no stride-0 stores
            # and why the first store is sliced).
            def dve_fill(dst, b, width):
                nc.vector.tensor_scalar(
                    out=dst,
                    in0=_rep_ap(s_v[:, b : b + 1], [[0, width]]),
                    scalar1=s_rr[:, b : b + 1],
                    scalar2=0.0,
                    op0=mybir.AluOpType.mult,
                    op1=mybir.AluOpType.max,
                )

            def act_fill(dst, b, width):
                nc.scalar.activation(
                    dst,
                    _rep_ap(s_v[:, BL + b : BL + b + 1], [[0, width]]),
                    mybir.ActivationFunctionType.Relu,
                    scale=s_rr[:, b : b + 1],
                )

            bc0 = sb.tile([128, 2 * P], f32, tag="bc0")
            bc1 = sb.tile([128, 2 * P], f32, tag="bc1")
            for lo, hi in zip(CUTS, CUTS[1:]):
                dve_fill(bc0[:, lo:hi], 0, hi - lo)
                nc.sync.dma_start(out=out_d[0][0:128, lo:hi], in_=bc0[:, lo:hi])
            act_fill(bc0[:, P : 2 * P], 0, P)
            nc.sync.dma_start(out=out_d[0][128:256, :], in_=bc0[:, P : 2 * P])
            dve_fill(bc1[:, 0:P], 1, P)
            nc.sync.dma_start(out=out_d[1][0:128, :], in_=bc1[:, 0:P])
            act_fill(bc1[:, P : 2 * P], 1, P)
            nc.sync.dma_start(out=out_d[1][128:256, :], in_=bc1[:, P : 2 * P])
    _fix_tail_drain(nc)
    return nc


def _fix_tail_drain(nc):
    """Walrus accepts very few sync waits per instruction, and Tile's
    kernel-tail drain waits on every semaphore. The whole dataflow funnels
    into the output DMAs, all FIFO on the sync queue, so waiting on the
    LAST one's completion sem alone is sufficient."""
    import bass_rust

    out_sem = None
    for ins in nc.inst_map.values():
        if type(ins).__name__ == "InstDMACopy" and "out_set" in str(ins):
            si = ins.sync_info
            if si is not None and len(si.on_update) > 0:
                out_sem = si.on_update[0].ant_name
    assert out_sem is not None, "output DMA completion sem not found"
    for ins in nc.inst_map.values():
        si = ins.sync_info
        if type(ins).__name__ == "InstDrain" and si is not None and len(si.on_wait) > 1:
            keep = [w for w in si.on_wait if w.ant_name == out_sem]
            assert len(keep) == 1, (out_sem, [w.ant_name for w in si.on_wait])
            ins.sync_info = bass_rust.SyncInfo(
                on_wait=keep, on_update=list(si.on_update)
            )


def _get_nc():
    if "nc" not in _cache:
        _cache["nc"] = _build_nc()
    return _cache["nc"]


def _to_bf16_pairs(a):
    """Round f32 array to bf16 (nearest-even) and pack pairs into f32 words:
    (..., 2k) f32 -> (..., k) f32 whose bytes are 2k bf16 values."""
    u = np.ascontiguousarray(a, dtype=np.float32).view(np.uint32)
    bf = ((u + 0x7FFF + ((u >> 16) & 1)) >> 16).astype(np.uint16)
    return bf.view(np.uint32).view(np.float32)


def _pack_cina(x_shard, nfh):
    cina = np.zeros((128, CINA_COLS), dtype=np.float32)
    x2 = x_shard.reshape(BN, HID)
    cina[0:BN, COL_X : COL_X + HID] = x2
    cina[0:BN, COL_NFH : COL_NFH + HID] = nfh[:, 0][None, :]
    for b in range(BL):
        cina[b * NODES : (b + 1) * NODES, COL_BM + b] = 1.0
    ones = np.ones((BN, 2), dtype=np.float32)
    cina[0:BN, COL_ONEC : COL_ONEC + 1] = _to_bf16_pairs(ones)
    cina[0:BN, COL_XBF : COL_XBF + HID // 2] = _to_bf16_pairs(x2)
    cina[0:1, COL_ONER : COL_ONER + 64] = _to_bf16_pairs(
        np.ones((1, 128), dtype=np.float32)
    )
    return cina


def _pack_cinb(w):
    # W[h, c] -> bf16[k, kh*256 + c] packed as f32 pairs (k = h % 128).
    wk = np.concatenate([w[0:128, :], w[128:256, :]], axis=1)  # (128, 512)
    return _to_bf16_pairs(wk)


def _make_in_maps(input, node_fea_for_hidden, weight):
    x_full = np.asarray(input, dtype=np.float32)[0]  # (B, N, HID)
    nfh = np.asarray(node_fea_for_hidden, dtype=np.float32)
    w = np.asarray(weight, dtype=np.float32)
    cinb = _pack_cinb(w)
    return [
        {"cina": _pack_cina(x_full[i * BL : (i + 1) * BL], nfh), "cinb": cinb}
        for i in range(NCORES)
    ]


def _run(in_maps, trace=False, **kwargs):
    nc = _get_nc()
    return run_bass_kernel_spmd(nc, in_maps, list(range(NCORES)), trace=trace, **kwargs)


def kernel(input, res_feature, node_fea_for_res, node_fea_for_hidden, weight):
    in_maps = _make_in_maps(input, node_fea_for_hidden, weight)
    res = _run(in_maps)
    shards = [res.results[i]["out"] for i in range(NCORES)]  # each (BL, C, P)
    full = np.concatenate(shards, axis=0)  # (B, C, P)
    return full.reshape(B, C, H, W).astype(np.float32, copy=False)
